# revision 28
# baseline (speedup 1.0000x reference)
"""MHSA Trainium2 Bass kernel.

Problem: B=4, P=4096, C=256, H=4 heads, D=64, fp32.
  q/k/v = x @ W{q,k,v} + b;  att = softmax(q k^T / sqrt(D)); out = (att v) @ Wo + bo

Sharding: 8 cores = (batch b, sequence half). Each core computes the full
attention output for 2048 query rows of one batch. K/V are computed on-core
from the full 4096-row x of that batch, so no collectives are needed. The
program is SPMD-uniform: query rows are always local rows 0..2048; for the
second half the host passes x rolled by -2048 rows (softmax over keys is
permutation invariant, so key order does not matter).

On-core pipeline (all matmuls in float32r: full PE rate at free-dim >= 256,
~1e-4 relative error):
  1. x -> x^T via PE transposes (c on partitions, 2 chunks of 128).
  2. Q^T, K^T (channel-major) and V (row-major) projections; biases fused
     into the PSUM->SBUF copies.  V is stored per (row-tile, head) with a
     65th column of ones: the ones column makes the PV matmul accumulate the
     softmax denominator as row 64 of the output.
  3. Flash loop per (q-512-tile m, head pair): S^T[keys,128 x m,512] tiles on
     PE (head pairs packed into disjoint PE row groups), exp on ACT
     (scale=1/sqrt(D) fused into the activation), unnormalized PV + denom
     accumulated in PSUM over all 32 key tiles.
  4. Normalize by 1/denom (DVE reciprocal + DMA partition-broadcast), then
     the Wo projection row-major and DMA out.
"""

import numpy as np

B, P, C, H, D = 4, 4096, 256, 4, 64
PQ = P // 2          # query rows per core
NPT = P // 128       # 32 key/row tiles
SCALE = float(D) ** -0.5
N_CORES = 8

_CACHE = {}


def _build():
    from contextlib import ExitStack

    import concourse.bass as bass
    import concourse.mybir as mybir
    import concourse.tile as tile
    from concourse import bacc
    from concourse.masks import make_identity

    def part_bcast(ap, parts):
        # replicate a [*free] AP across `parts` partitions (DMA replication)
        return bass.AP(tensor=ap.tensor, offset=ap.offset, ap=[[0, parts]] + list(ap.ap))

    F32 = mybir.dt.float32
    F32R = mybir.dt.float32r
    BF16 = mybir.dt.bfloat16
    EXP = mybir.ActivationFunctionType.Exp

    nc = bacc.Bacc("TRN2", target_bir_lowering=False, debug=False)

    x_d = nc.dram_tensor("x", [P, C], F32, kind="ExternalInput")
    w_d = {
        nm: nc.dram_tensor(nm, [C, C], BF16, kind="ExternalInput")
        for nm in ("Wq", "Wk", "Wv", "Wo")
    }
    b_d = {
        nm: nc.dram_tensor(nm, [C], F32, kind="ExternalInput")
        for nm in ("bq", "bk", "bv", "bo")
    }
    out_d = nc.dram_tensor("out", [PQ, C], F32, kind="ExternalOutput")

    with tile.TileContext(nc) as tc, ExitStack() as ctx:
        const = ctx.enter_context(tc.tile_pool(name="const", bufs=1))
        big = ctx.enter_context(tc.tile_pool(name="big", bufs=1))
        ptiles = ctx.enter_context(tc.tile_pool(name="ptiles", bufs=2))
        stage = ctx.enter_context(tc.tile_pool(name="stage", bufs=3))
        small = ctx.enter_context(tc.tile_pool(name="small", bufs=4))

        ident_f = const.tile([128, 128], F32, tag="ident_f")
        make_identity(nc, ident_f)
        ident = const.tile([128, 128], F32R, tag="ident")
        nc.vector.tensor_copy(out=ident, in_=ident_f)
        ones_row = const.tile([1, 64], F32R, tag="ones_row")
        nc.gpsimd.memset(ones_row[:].bitcast(F32), 1.0)

        # touch exp early so the ACT table set loads during phase 1, not at
        # the first real softmax tile (table load is ~2.7us of ACT stall)
        warm = const.tile([1, 8], F32, tag="act_warm")
        nc.gpsimd.memset(warm, 0.0)
        nc.scalar.activation(out=warm, in_=warm, func=EXP)

        # weight/bias loads are deferred until after the first x-tile DMA is
        # enqueued, so the transpose pipeline starts as early as possible
        w_sb, bias_sb, bcast_sb = {}, {}, {}

        def load_weights():
            for nm in ("Wk", "Wq", "Wv", "Wo"):
                t = const.tile([128, 2, C], BF16, tag=f"w_{nm}", name=f"w_{nm}")
                for c2 in range(2):
                    nc.sync.dma_start(
                        out=t[:, c2, :],
                        in_=w_d[nm][c2 * 128 : (c2 + 1) * 128, :],
                    )
                w_sb[nm] = t
            # per-partition bias layout for the channel-major Q^T/K^T tiles
            for nm in ("bq", "bk"):
                t = const.tile([128, 2], F32, tag=f"b_{nm}", name=f"b_{nm}")
                nc.sync.dma_start(
                    out=t, in_=b_d[nm][:].rearrange("(c p) -> p c", p=128)
                )
                bias_sb[nm] = t
            # row-broadcast bias tiles for the row-major V / final projections
            for nm in ("bv", "bo"):
                t = const.tile([128, C], F32, tag=f"b_{nm}", name=f"b_{nm}")
                nc.gpsimd.dma_start(out=t, in_=part_bcast(b_d[nm][:], 128))
                bcast_sb[nm] = t

        xT = big.tile([128, 2, P], BF16, tag="xT")
        QT = big.tile([128, 2, PQ], BF16, tag="QT")
        KT = big.tile([128, 2, P], BF16, tag="KT")
        Vp = big.tile([128, NPT, H, D + 1], BF16, tag="Vp")
        OT = big.tile([128, 2, PQ], BF16, tag="OT")

        # ones column used by the PV matmul to accumulate softmax denominators
        nc.gpsimd.memset(Vp[:, :, :, D : D + 1], 1.0)

        # ---- phase 1: x^T, Q^T, K^T, V (interleaved per x-tile group) ----
        # Per 128-row x tile: DMA -> PE transpose -> V projection; after each
        # 512-row group, the K (and Q) projections for that group. Keeps the
        # PE stream dense (HAM-warm) instead of three serial sweeps, and gets
        # the first flash-loop inputs ready as early as possible.
        COPY = mybir.ActivationFunctionType.Identity
        with (
            tc.tile_pool(name="ps_tr", bufs=2, space="PSUM") as ps_tr,
            tc.tile_pool(name="ps_pj", bufs=2, space="PSUM") as ps_pj,
        ):
            def proj_kq(dst, w, bias, c2, mt):
                pp = ps_pj.tile([128, 512], F32, tag="proj", name="pp")
                for ci in range(2):
                    nc.tensor.matmul(
                        pp,
                        lhsT=w[:, ci, c2 * 128 : (c2 + 1) * 128],
                        rhs=xT[:, ci, mt * 512 : (mt + 1) * 512],
                        start=(ci == 0),
                        stop=(ci == 1),
                    )
                # evacuate on ACT (idle during phase 1; DVE is the phase-1
                # pacer): out = pp*1 + bias[c]
                nc.scalar.activation(
                    out=dst[:, c2, mt * 512 : (mt + 1) * 512],
                    in_=pp,
                    func=COPY,
                    bias=bias[:, c2 : c2 + 1],
                )

            for pt in range(NPT):
                xt = stage.tile([128, C], F32R, tag="xin")
                nc.sync.dma_start(
                    out=xt, in_=x_d[pt * 128 : (pt + 1) * 128, :].bitcast(F32R)
                )
                if pt == 0:
                    load_weights()
                for c2 in range(2):
                    tp = ps_tr.tile([128, 128], F32R, tag="tr")
                    nc.tensor.transpose(tp, xt[:, c2 * 128 : (c2 + 1) * 128], ident)
                    nc.vector.tensor_copy(
                        out=xT[:, c2, pt * 128 : (pt + 1) * 128],
                        in_=tp.bitcast(F32),
                    )
                pv = ps_pj.tile([128, C], F32, tag="vproj", name="pv")
                for ci in range(2):
                    nc.tensor.matmul(
                        pv,
                        lhsT=xT[:, ci, pt * 128 : (pt + 1) * 128],
                        rhs=w_sb["Wv"][:, ci, :],
                        start=(ci == 0),
                        stop=(ci == 1),
                    )
                nc.vector.tensor_add(
                    out=Vp[:, pt, :, 0:D],
                    in0=pv.rearrange("p (h d) -> p h d", d=D),
                    in1=bcast_sb["bv"].rearrange("p (h d) -> p h d", d=D),
                )
                if pt % 4 == 3:
                    mt = pt // 4
                    for c2 in range(2):
                        proj_kq(KT, w_sb["Wk"], bias_sb["bk"], c2, mt)
                        if mt < PQ // 512:
                            proj_kq(QT, w_sb["Wq"], bias_sb["bq"], c2, mt)
                if pt == 3:
                    # early HAM warm-up: dense back-to-back matmuls right after
                    # the first projection group unthrottle the PE for the rest
                    # of phase 1
                    with tc.tile_critical():
                        hamp = ps_pj.tile([128, 512], F32, tag="hamp", name="hamp")
                        for _ in range(10):
                            nc.tensor.matmul(
                                hamp,
                                lhsT=w_sb["Wk"][:, 0, 0:128],
                                rhs=xT[:, 0, 0:512],
                                start=True,
                                stop=True,
                            )

        # ---- phase 2: attention + output projection ----
        # The per-(m,pair) epilogue (reciprocal of the softmax denominator,
        # partition-broadcast, normalize, Wo projection, output DMA) has a
        # long serial latency chain (the [1,512] DVE reciprocal alone is
        # ~3.3us). If emitted in program order it stalls the PE at every
        # tile boundary, which re-throttles the HAM clock gate to 1.2 GHz
        # for the following stretch. Instead: evacuate the o PSUM banks
        # with a cheap DVE copy right away, then defer the rest of the
        # epilogue and emit it interleaved into the NEXT (m,pair)'s flash
        # loop so every engine always has ready work.
        with (
            tc.tile_pool(name="ps_s", bufs=1, space="PSUM") as ps_s,
            tc.tile_pool(name="ps_o", bufs=1, space="PSUM") as ps_o,
            tc.tile_pool(name="ps_w", bufs=1, space="PSUM") as ps_w,
            tc.tile_pool(name="obuf", bufs=2) as obuf,
        ):
            pending = []  # deferred epilogue thunks [(phase, fn), ...]

            def flush(phase):
                for ph, fn in [p for p in pending if p[0] == phase]:
                    fn()
                pending[:] = [p for p in pending if p[0] != phase]

            def epilogue_norm(m, ob_pair, heads):
                # reciprocal + broadcast + normalize into OT (DVE/PE work,
                # dependencies are several us old by the time this runs)
                for j, h in enumerate(heads):
                    ob = ob_pair[j]
                    rc = small.tile([1, 512], F32R, tag="recip")
                    with nc.allow_low_precision(reason="f32r recip ~1e-5"):
                        nc.vector.reciprocal(out=rc, in_=ob[D : D + 1, :])
                    bc = ps_w.tile([64, 512], F32, tag="rbc")
                    nc.tensor.matmul(bc, lhsT=ones_row, rhs=rc, start=True, stop=True)
                    bcs = small.tile([64, 512], F32, tag="bcs")
                    nc.vector.tensor_copy(out=bcs, in_=bc)
                    bp, ch = 64 * (h % 2), h // 2
                    nc.vector.tensor_mul(
                        out=OT[bp : bp + 64, ch, m * 512 : (m + 1) * 512],
                        in0=ob[0:D, :],
                        in1=bcs,
                    )

            def epilogue_wo(m):
                for pt4 in range(4):
                    pi = m * 4 + pt4
                    wp = ps_w.tile([128, C], F32, tag="wo")
                    for ci in range(2):
                        nc.tensor.matmul(
                            wp,
                            lhsT=OT[:, ci, pi * 128 : (pi + 1) * 128],
                            rhs=w_sb["Wo"][:, ci, :],
                            start=(ci == 0),
                            stop=(ci == 1),
                        )
                    ot = stage.tile([128, C], F32, tag="outt")
                    nc.vector.tensor_add(out=ot, in0=wp, in1=bcast_sb["bo"])
                    nc.sync.dma_start(out=out_d[pi * 128 : (pi + 1) * 128, :], in_=ot)

            NG = NPT // 2

            def emit_unit(m, heads, g):
                # one S+exp unit: 4 score matmuls (head pair row-grouped) for
                # key tiles 2g, 2g+1, plus the two exp activations
                s_ps = [
                    ps_s.tile([128, 2, 512], F32, tag=f"s{j}", name=f"s{j}")
                    for j in range(2)
                ]
                for j2 in range(2):
                    kt = 2 * g + j2
                    for j, h in enumerate(heads):
                        bp, ch = 64 * (h % 2), h // 2
                        nc.tensor.matmul(
                            s_ps[j][:, j2, :],
                            lhsT=KT[bp : bp + 64, ch, kt * 128 : (kt + 1) * 128],
                            rhs=QT[bp : bp + 64, ch, m * 512 : (m + 1) * 512],
                            start=True,
                            stop=True,
                        )
                p_sb = [
                    ptiles.tile([128, 2, 512], BF16, tag=f"p{j}", name=f"p{j}")
                    for j in range(2)
                ]
                for j in range(2):
                    nc.scalar.activation(out=p_sb[j], in_=s_ps[j], func=EXP, scale=SCALE)
                return p_sb

            # The S matmuls for unit i+2 are emitted BEFORE the PV matmuls of
            # unit i — across pair boundaries too — so in the PE's FIFO the
            # next scores are always queued ahead: the ACT engine never waits
            # for score tiles and runs gapless, which is the phase-2 critical
            # path.
            SEQ = [(m, pair) for m in range(PQ // 512) for pair in range(2)]
            TOT = len(SEQ) * NG

            def ctx_of(i):
                m, pair = SEQ[i // NG]
                return m, pair, i % NG

            def unit_of(i):
                m, pair, g = ctx_of(i)
                return emit_unit(m, (2 * pair, 2 * pair + 1), g)

            units = {0: unit_of(0), 1: unit_of(1)}
            o_ps = None
            for i in range(TOT):
                m, pair, g = ctx_of(i)
                heads = (2 * pair, 2 * pair + 1)
                p_sb = units.pop(i)
                if g == 0:
                    o_ps = [
                        ps_o.tile([D + 1, 512], F32, tag=f"o{j}", name=f"o{j}")
                        for j in range(2)
                    ]
                if i == 3:
                    # ~5us of dense back-to-back matmuls: guarantees a
                    # fully-busy HAM SHORT window so the PE un-throttles to
                    # 2.4 GHz early. lhsT depends on this unit's exp output,
                    # which pins the burst into the loop's timeline; the
                    # critical section keeps it contiguous (the scheduler
                    # would otherwise sink this no-consumer work to the end).
                    with tc.tile_critical():
                        hamw = ps_w.tile([128, 512], F32, tag="rbc", name="hamw")
                        for _ in range(12):
                            nc.tensor.matmul(
                                hamw,
                                lhsT=p_sb[0][:, 0, 0:128],
                                rhs=xT[:, 0, 0:512],
                                start=True,
                                stop=True,
                            )
                if g == 4:
                    flush("norm")
                if g == 8:
                    flush("wo")
                if g in (0, 15) and i > 3:
                    # pair-boundary HAM pad: keeps the PE idle window at the
                    # epilogue handoff below the re-throttle threshold.
                    padw = ps_w.tile([128, 512], F32, tag="rbc", name=f"padw{g}")
                    for _ in range(3):
                        nc.tensor.matmul(
                            padw,
                            lhsT=p_sb[0][:, 0, 0:128],
                            rhs=xT[:, 0, 0:512],
                            start=True,
                            stop=True,
                        )
                for j2 in range(2):
                    kt = 2 * g + j2
                    for j, h in enumerate(heads):
                        nc.tensor.matmul(
                            o_ps[j],
                            lhsT=Vp[:, kt, h, :],
                            rhs=p_sb[j][:, j2, :],
                            start=(kt == 0),
                            stop=(kt == NPT - 1),
                            skip_group_check=True,
                        )
                if i + 2 < TOT:
                    units[i + 2] = unit_of(i + 2)
                if g == NG - 1:
                    # evacuate o PSUM banks immediately (cheap copies) so the
                    # next pair's PV matmuls are never blocked on the slow
                    # normalization chain
                    ob_pair = [
                        obuf.tile([D + 1, 512], F32, tag=f"ob{j}", name=f"ob{j}")
                        for j in range(2)
                    ]
                    for j in range(2):
                        nc.vector.tensor_copy(out=ob_pair[j], in_=o_ps[j])
                    pending.append(
                        ("norm", (lambda mm, op, hh: lambda: epilogue_norm(mm, op, hh))(m, ob_pair, heads))
                    )
                    if pair == 1:
                        pending.append(("wo", (lambda mm: lambda: epilogue_wo(mm))(m)))
            flush("norm")
            flush("wo")

    nc.compile()
    return nc


def _get_nc():
    if "nc" not in _CACHE:
        _CACHE["nc"] = _build()
    return _CACHE["nc"]


def _in_maps(inputs):
    import ml_dtypes

    x = np.ascontiguousarray(np.asarray(inputs["x"], dtype=np.float32))
    assert x.shape == (B, P, C), x.shape
    shared = {}
    for nm in ("bq", "bk", "bv", "bo"):
        shared[nm] = np.ascontiguousarray(np.asarray(inputs[nm], dtype=np.float32))
    for nm in ("Wq", "Wk", "Wv", "Wo"):
        shared[nm] = np.ascontiguousarray(
            np.asarray(inputs[nm], dtype=np.float32).astype(ml_dtypes.bfloat16)
        )
    maps = []
    for core in range(N_CORES):
        b, half = core // 2, core % 2
        if half == 0:
            xl = np.ascontiguousarray(x[b])
        else:
            xl = np.ascontiguousarray(np.roll(x[b], -PQ, axis=0))
        maps.append({"x": xl, **shared})
    return maps


def run(inputs, trace=False):
    from concourse import bass_utils

    nc = _get_nc()
    res = bass_utils.run_bass_kernel_spmd(
        nc, _in_maps(inputs), core_ids=list(range(N_CORES)), trace=trace
    )
    out = np.empty((B, P, C), np.float32)
    for core in range(N_CORES):
        b, half = core // 2, core % 2
        out[b, half * PQ : (half + 1) * PQ] = res.results[core]["out"]
    return out, res


def kernel(**inputs):
    out, _ = run(inputs, trace=False)
    return out



# revision 31
# speedup vs baseline: 1.1094x; 1.1094x over previous
"""MHSA Trainium2 Bass kernel.

Problem: B=4, P=4096, C=256, H=4 heads, D=64, fp32.
  q/k/v = x @ W{q,k,v} + b;  att = softmax(q k^T / sqrt(D)); out = (att v) @ Wo + bo

Sharding: 8 cores = (batch b, sequence half). Each core computes the full
attention output for 2048 query rows of one batch. K/V are computed on-core
from the full 4096-row x of that batch, so no collectives are needed. The
program is SPMD-uniform: query rows are always local rows 0..2048; for the
second half the host passes x rolled by -2048 rows (softmax over keys is
permutation invariant, so key order does not matter).

On-core pipeline (all matmuls in float32r: full PE rate at free-dim >= 256,
~1e-4 relative error):
  1. x -> x^T via PE transposes (c on partitions, 2 chunks of 128).
  2. Q^T, K^T (channel-major) and V (row-major) projections; biases fused
     into the PSUM->SBUF copies.  V is stored per (row-tile, head) with a
     65th column of ones: the ones column makes the PV matmul accumulate the
     softmax denominator as row 64 of the output.
  3. Flash loop per (q-512-tile m, head pair): S^T[keys,128 x m,512] tiles on
     PE (head pairs packed into disjoint PE row groups), exp on ACT
     (scale=1/sqrt(D) fused into the activation), unnormalized PV + denom
     accumulated in PSUM over all 32 key tiles.
  4. Normalize by 1/denom (DVE reciprocal + DMA partition-broadcast), then
     the Wo projection row-major and DMA out.
"""

import numpy as np

B, P, C, H, D = 4, 4096, 256, 4, 64
PQ = P // 2          # query rows per core
NPT = P // 128       # 32 key/row tiles
SCALE = float(D) ** -0.5
N_CORES = 8

_CACHE = {}


def _build():
    from contextlib import ExitStack

    import concourse.bass as bass
    import concourse.mybir as mybir
    import concourse.tile as tile
    from concourse import bacc
    from concourse.masks import make_identity

    def part_bcast(ap, parts):
        # replicate a [*free] AP across `parts` partitions (DMA replication)
        return bass.AP(tensor=ap.tensor, offset=ap.offset, ap=[[0, parts]] + list(ap.ap))

    F32 = mybir.dt.float32
    F32R = mybir.dt.float32r
    BF16 = mybir.dt.bfloat16
    EXP = mybir.ActivationFunctionType.Exp

    nc = bacc.Bacc("TRN2", target_bir_lowering=False, debug=False)

    x_d = nc.dram_tensor("x", [P, C], F32, kind="ExternalInput")
    w_d = {
        nm: nc.dram_tensor(nm, [C, C], BF16, kind="ExternalInput")
        for nm in ("Wq", "Wk", "Wv", "Wo")
    }
    b_d = {
        nm: nc.dram_tensor(nm, [C], F32, kind="ExternalInput")
        for nm in ("bq", "bk", "bv", "bo")
    }
    out_d = nc.dram_tensor("out", [PQ, C], F32, kind="ExternalOutput")

    with tile.TileContext(nc) as tc, ExitStack() as ctx:
        const = ctx.enter_context(tc.tile_pool(name="const", bufs=1))
        big = ctx.enter_context(tc.tile_pool(name="big", bufs=1))
        ptiles = ctx.enter_context(tc.tile_pool(name="ptiles", bufs=2))
        stage = ctx.enter_context(tc.tile_pool(name="stage", bufs=3))
        small = ctx.enter_context(tc.tile_pool(name="small", bufs=4))

        ident_f = const.tile([128, 128], F32, tag="ident_f")
        make_identity(nc, ident_f)
        ident = const.tile([128, 128], F32R, tag="ident")
        nc.vector.tensor_copy(out=ident, in_=ident_f)
        ones_row = const.tile([1, 64], F32R, tag="ones_row")
        nc.gpsimd.memset(ones_row[:].bitcast(F32), 1.0)

        # touch exp early so the ACT table set loads during phase 1, not at
        # the first real softmax tile (table load is ~2.7us of ACT stall)
        warm = const.tile([1, 8], F32, tag="act_warm")
        nc.gpsimd.memset(warm, 0.0)
        nc.scalar.activation(out=warm, in_=warm, func=EXP)

        # weight/bias loads are deferred until after the first x-tile DMA is
        # enqueued, so the transpose pipeline starts as early as possible
        w_sb, bias_sb, bcast_sb = {}, {}, {}

        def load_weights():
            for nm in ("Wk", "Wq", "Wv", "Wo"):
                t = const.tile([128, 2, C], BF16, tag=f"w_{nm}", name=f"w_{nm}")
                for c2 in range(2):
                    nc.sync.dma_start(
                        out=t[:, c2, :],
                        in_=w_d[nm][c2 * 128 : (c2 + 1) * 128, :],
                    )
                w_sb[nm] = t
            # per-partition bias layout for the channel-major Q^T/K^T tiles
            for nm in ("bq", "bk"):
                t = const.tile([128, 2], F32, tag=f"b_{nm}", name=f"b_{nm}")
                nc.sync.dma_start(
                    out=t, in_=b_d[nm][:].rearrange("(c p) -> p c", p=128)
                )
                bias_sb[nm] = t
            # row-broadcast bias tiles for the row-major V / final projections
            for nm in ("bv", "bo"):
                t = const.tile([128, C], F32, tag=f"b_{nm}", name=f"b_{nm}")
                nc.gpsimd.dma_start(out=t, in_=part_bcast(b_d[nm][:], 128))
                bcast_sb[nm] = t

        xT = big.tile([128, 2, P], BF16, tag="xT")
        QT = big.tile([128, 2, PQ], BF16, tag="QT")
        KT = big.tile([128, 2, P], BF16, tag="KT")
        Vp = big.tile([128, NPT, H, D + 1], BF16, tag="Vp")
        OT = big.tile([128, 2, PQ], BF16, tag="OT")

        # ones column used by the PV matmul to accumulate softmax denominators
        nc.gpsimd.memset(Vp[:, :, :, D : D + 1], 1.0)

        # ---- phase 1: x^T, Q^T, K^T, V (interleaved per x-tile group) ----
        # Per 128-row x tile: DMA -> PE transpose -> V projection; after each
        # 512-row group, the K (and Q) projections for that group. Keeps the
        # PE stream dense (HAM-warm) instead of three serial sweeps, and gets
        # the first flash-loop inputs ready as early as possible.
        COPY = mybir.ActivationFunctionType.Identity
        with (
            tc.tile_pool(name="ps_tr", bufs=2, space="PSUM") as ps_tr,
            tc.tile_pool(name="ps_pj", bufs=2, space="PSUM") as ps_pj,
        ):
            def proj_kq(dst, w, bias, c2, mt):
                pp = ps_pj.tile([128, 512], F32, tag="proj", name="pp")
                for ci in range(2):
                    nc.tensor.matmul(
                        pp,
                        lhsT=w[:, ci, c2 * 128 : (c2 + 1) * 128],
                        rhs=xT[:, ci, mt * 512 : (mt + 1) * 512],
                        start=(ci == 0),
                        stop=(ci == 1),
                    )
                # evacuate on ACT (idle during phase 1; DVE is the phase-1
                # pacer): out = pp*1 + bias[c]
                nc.scalar.activation(
                    out=dst[:, c2, mt * 512 : (mt + 1) * 512],
                    in_=pp,
                    func=COPY,
                    bias=bias[:, c2 : c2 + 1],
                )

            for pt in range(NPT):
                xt = stage.tile([128, C], F32R, tag="xin")
                nc.sync.dma_start(
                    out=xt, in_=x_d[pt * 128 : (pt + 1) * 128, :].bitcast(F32R)
                )
                if pt == 0:
                    load_weights()
                for c2 in range(2):
                    tp = ps_tr.tile([128, 128], F32R, tag="tr")
                    nc.tensor.transpose(tp, xt[:, c2 * 128 : (c2 + 1) * 128], ident)
                    nc.vector.tensor_copy(
                        out=xT[:, c2, pt * 128 : (pt + 1) * 128],
                        in_=tp.bitcast(F32),
                    )
                pv = ps_pj.tile([128, C], F32, tag="vproj", name="pv")
                for ci in range(2):
                    nc.tensor.matmul(
                        pv,
                        lhsT=xT[:, ci, pt * 128 : (pt + 1) * 128],
                        rhs=w_sb["Wv"][:, ci, :],
                        start=(ci == 0),
                        stop=(ci == 1),
                    )
                nc.vector.tensor_add(
                    out=Vp[:, pt, :, 0:D],
                    in0=pv.rearrange("p (h d) -> p h d", d=D),
                    in1=bcast_sb["bv"].rearrange("p (h d) -> p h d", d=D),
                )
                if pt % 4 == 3:
                    mt = pt // 4
                    for c2 in range(2):
                        proj_kq(KT, w_sb["Wk"], bias_sb["bk"], c2, mt)
                        if mt < PQ // 512:
                            proj_kq(QT, w_sb["Wq"], bias_sb["bq"], c2, mt)
                if pt == 3:
                    # early HAM warm-up: dense back-to-back matmuls right after
                    # the first projection group unthrottle the PE for the rest
                    # of phase 1
                    with tc.tile_critical():
                        hamp = ps_pj.tile([128, 512], F32, tag="hamp", name="hamp")
                        for _ in range(10):
                            nc.tensor.matmul(
                                hamp,
                                lhsT=w_sb["Wk"][:, 0, 0:128],
                                rhs=xT[:, 0, 0:512],
                                start=True,
                                stop=True,
                            )

        # ---- phase 2: attention + output projection ----
        # The per-(m,pair) epilogue (reciprocal of the softmax denominator,
        # partition-broadcast, normalize, Wo projection, output DMA) has a
        # long serial latency chain (the [1,512] DVE reciprocal alone is
        # ~3.3us). If emitted in program order it stalls the PE at every
        # tile boundary, which re-throttles the HAM clock gate to 1.2 GHz
        # for the following stretch. Instead: evacuate the o PSUM banks
        # with a cheap DVE copy right away, then defer the rest of the
        # epilogue and emit it interleaved into the NEXT (m,pair)'s flash
        # loop so every engine always has ready work.
        with (
            tc.tile_pool(name="ps_s", bufs=1, space="PSUM") as ps_s,
            tc.tile_pool(name="ps_o", bufs=1, space="PSUM") as ps_o,
            tc.tile_pool(name="ps_w", bufs=1, space="PSUM") as ps_w,
            tc.tile_pool(name="obuf", bufs=2) as obuf,
        ):
            pending = []  # deferred epilogue thunks [(phase, fn), ...]

            def flush(phase):
                for ph, fn in [p for p in pending if p[0] == phase]:
                    fn()
                pending[:] = [p for p in pending if p[0] != phase]

            def epilogue_norm(m, ob_pair, heads, q0=0, q1=512):
                # reciprocal + broadcast + normalize into OT (DVE/PE work,
                # dependencies are several us old by the time this runs)
                qn = q1 - q0
                for j, h in enumerate(heads):
                    ob = ob_pair[j]
                    rc = small.tile([1, qn], F32R, tag="recip", name="rc")
                    with nc.allow_low_precision(reason="f32r recip ~1e-5"):
                        nc.vector.reciprocal(out=rc, in_=ob[D : D + 1, q0:q1])
                    bc = ps_w.tile([64, qn], F32, tag="rbc", name="bc")
                    nc.tensor.matmul(bc, lhsT=ones_row, rhs=rc, start=True, stop=True)
                    bcs = small.tile([64, qn], F32, tag="bcs", name="bcs")
                    nc.vector.tensor_copy(out=bcs, in_=bc)
                    bp, ch = 64 * (h % 2), h // 2
                    nc.vector.tensor_mul(
                        out=OT[bp : bp + 64, ch, m * 512 + q0 : m * 512 + q1],
                        in0=ob[0:D, q0:q1],
                        in1=bcs,
                    )

            def epilogue_wo(m, c0=0, c1=4):
                for pt4 in range(c0, c1):
                    pi = m * 4 + pt4
                    wp = ps_w.tile([128, C], F32, tag="wo", name="wp")
                    for ci in range(2):
                        nc.tensor.matmul(
                            wp,
                            lhsT=OT[:, ci, pi * 128 : (pi + 1) * 128],
                            rhs=w_sb["Wo"][:, ci, :],
                            start=(ci == 0),
                            stop=(ci == 1),
                        )
                    ot = stage.tile([128, C], F32, tag="outt", name="ot")
                    nc.vector.tensor_add(out=ot, in0=wp, in1=bcast_sb["bo"])
                    nc.sync.dma_start(out=out_d[pi * 128 : (pi + 1) * 128, :], in_=ot)

            NG = NPT // 2

            def emit_unit(m, heads, g):
                # one S+exp unit: 4 score matmuls (head pair row-grouped) for
                # key tiles 2g, 2g+1, plus the two exp activations
                s_ps = [
                    ps_s.tile([128, 2, 512], F32, tag=f"s{j}", name=f"s{j}")
                    for j in range(2)
                ]
                for j2 in range(2):
                    kt = 2 * g + j2
                    for j, h in enumerate(heads):
                        bp, ch = 64 * (h % 2), h // 2
                        nc.tensor.matmul(
                            s_ps[j][:, j2, :],
                            lhsT=KT[bp : bp + 64, ch, kt * 128 : (kt + 1) * 128],
                            rhs=QT[bp : bp + 64, ch, m * 512 : (m + 1) * 512],
                            start=True,
                            stop=True,
                        )
                p_sb = [
                    ptiles.tile([128, 2, 512], BF16, tag=f"p{j}", name=f"p{j}")
                    for j in range(2)
                ]
                for j in range(2):
                    nc.scalar.activation(out=p_sb[j], in_=s_ps[j], func=EXP, scale=SCALE)
                return p_sb

            # The S matmuls for unit i+2 are emitted BEFORE the PV matmuls of
            # unit i — across pair boundaries too — so in the PE's FIFO the
            # next scores are always queued ahead: the ACT engine never waits
            # for score tiles and runs gapless, which is the phase-2 critical
            # path.
            SEQ = [(m, pair) for m in range(PQ // 512) for pair in range(2)]
            TOT = len(SEQ) * NG

            def ctx_of(i):
                m, pair = SEQ[i // NG]
                return m, pair, i % NG

            def unit_of(i):
                m, pair, g = ctx_of(i)
                return emit_unit(m, (2 * pair, 2 * pair + 1), g)

            units = {0: unit_of(0), 1: unit_of(1)}
            o_ps = None
            for i in range(TOT):
                m, pair, g = ctx_of(i)
                heads = (2 * pair, 2 * pair + 1)
                p_sb = units.pop(i)
                if g == 0:
                    o_ps = [
                        ps_o.tile([D + 1, 512], F32, tag=f"o{j}", name=f"o{j}")
                        for j in range(2)
                    ]
                if i == 3:
                    # ~5us of dense back-to-back matmuls: guarantees a
                    # fully-busy HAM SHORT window so the PE un-throttles to
                    # 2.4 GHz early. lhsT depends on this unit's exp output,
                    # which pins the burst into the loop's timeline; the
                    # critical section keeps it contiguous (the scheduler
                    # would otherwise sink this no-consumer work to the end).
                    with tc.tile_critical():
                        hamw = ps_w.tile([128, 512], F32, tag="rbc", name="hamw")
                        for _ in range(12):
                            nc.tensor.matmul(
                                hamw,
                                lhsT=p_sb[0][:, 0, 0:128],
                                rhs=xT[:, 0, 0:512],
                                start=True,
                                stop=True,
                            )
                if g == 4:
                    flush("norm")
                if g == 8:
                    flush("wo")
                if g in (0, 1, 14, 15) and i > 3:
                    # pair-boundary HAM pad: keeps the PE idle window at the
                    # epilogue handoff below the re-throttle threshold.
                    padw = ps_w.tile([128, 512], F32, tag="rbc", name=f"padw{g}")
                    for _ in range(3):
                        nc.tensor.matmul(
                            padw,
                            lhsT=p_sb[0][:, 0, 0:128],
                            rhs=xT[:, 0, 0:512],
                            start=True,
                            stop=True,
                        )
                for j2 in range(2):
                    kt = 2 * g + j2
                    for j, h in enumerate(heads):
                        nc.tensor.matmul(
                            o_ps[j],
                            lhsT=Vp[:, kt, h, :],
                            rhs=p_sb[j][:, j2, :],
                            start=(kt == 0),
                            stop=(kt == NPT - 1),
                            skip_group_check=True,
                        )
                if i + 2 < TOT:
                    units[i + 2] = unit_of(i + 2)
                if g == NG - 1:
                    # evacuate o PSUM banks immediately (cheap copies) so the
                    # next pair's PV matmuls are never blocked on the slow
                    # normalization chain
                    ob_pair = [
                        obuf.tile([D + 1, 512], F32, tag=f"ob{j}", name=f"ob{j}")
                        for j in range(2)
                    ]
                    for j in range(2):
                        nc.vector.tensor_copy(out=ob_pair[j], in_=o_ps[j])
                    if i == TOT - 1:
                        # final pair: emit the epilogue chunked by query-half
                        # so the reciprocal / broadcast / Wo / DMA chains of
                        # the two halves pipeline, shortening the kernel tail
                        for half in range(2):
                            epilogue_norm(m, ob_pair, heads, half * 256, (half + 1) * 256)
                            epilogue_wo(m, half * 2, (half + 1) * 2)
                    else:
                        pending.append(
                            ("norm", (lambda mm, op, hh: lambda: epilogue_norm(mm, op, hh))(m, ob_pair, heads))
                        )
                        if pair == 1:
                            pending.append(("wo", (lambda mm: lambda: epilogue_wo(mm))(m)))
            flush("norm")
            flush("wo")

    nc.compile()
    return nc


def _get_nc():
    if "nc" not in _CACHE:
        _CACHE["nc"] = _build()
    return _CACHE["nc"]


def _in_maps(inputs):
    import ml_dtypes

    x = np.ascontiguousarray(np.asarray(inputs["x"], dtype=np.float32))
    assert x.shape == (B, P, C), x.shape
    shared = {}
    for nm in ("bq", "bk", "bv", "bo"):
        shared[nm] = np.ascontiguousarray(np.asarray(inputs[nm], dtype=np.float32))
    for nm in ("Wq", "Wk", "Wv", "Wo"):
        shared[nm] = np.ascontiguousarray(
            np.asarray(inputs[nm], dtype=np.float32).astype(ml_dtypes.bfloat16)
        )
    maps = []
    for core in range(N_CORES):
        b, half = core // 2, core % 2
        if half == 0:
            xl = np.ascontiguousarray(x[b])
        else:
            xl = np.ascontiguousarray(np.roll(x[b], -PQ, axis=0))
        maps.append({"x": xl, **shared})
    return maps


def run(inputs, trace=False):
    from concourse import bass_utils

    nc = _get_nc()
    res = bass_utils.run_bass_kernel_spmd(
        nc, _in_maps(inputs), core_ids=list(range(N_CORES)), trace=trace
    )
    out = np.empty((B, P, C), np.float32)
    for core in range(N_CORES):
        b, half = core // 2, core % 2
        out[b, half * PQ : (half + 1) * PQ] = res.results[core]["out"]
    return out, res


def kernel(**inputs):
    out, _ = run(inputs, trace=False)
    return out



# revision 38
# speedup vs baseline: 1.1160x; 1.0059x over previous
"""MHSA Trainium2 Bass kernel.

Problem: B=4, P=4096, C=256, H=4 heads, D=64, fp32.
  q/k/v = x @ W{q,k,v} + b;  att = softmax(q k^T / sqrt(D)); out = (att v) @ Wo + bo

Sharding: 8 cores = (batch b, sequence half). Each core computes the full
attention output for 2048 query rows of one batch. K/V are computed on-core
from the full 4096-row x of that batch, so no collectives are needed. The
program is SPMD-uniform: query rows are always local rows 0..2048; for the
second half the host passes x rolled by -2048 rows (softmax over keys is
permutation invariant, so key order does not matter).

On-core pipeline (all matmuls in float32r: full PE rate at free-dim >= 256,
~1e-4 relative error):
  1. x -> x^T via PE transposes (c on partitions, 2 chunks of 128).
  2. Q^T, K^T (channel-major) and V (row-major) projections; biases fused
     into the PSUM->SBUF copies.  V is stored per (row-tile, head) with a
     65th column of ones: the ones column makes the PV matmul accumulate the
     softmax denominator as row 64 of the output.
  3. Flash loop per (q-512-tile m, head pair): S^T[keys,128 x m,512] tiles on
     PE (head pairs packed into disjoint PE row groups), exp on ACT
     (scale=1/sqrt(D) fused into the activation), unnormalized PV + denom
     accumulated in PSUM over all 32 key tiles.
  4. Normalize by 1/denom (DVE reciprocal + DMA partition-broadcast), then
     the Wo projection row-major and DMA out.
"""

import numpy as np

B, P, C, H, D = 4, 4096, 256, 4, 64
PQ = P // 2          # query rows per core
NPT = P // 128       # 32 key/row tiles
SCALE = float(D) ** -0.5
N_CORES = 8

_CACHE = {}


def _build():
    from contextlib import ExitStack

    import concourse.bass as bass
    import concourse.mybir as mybir
    import concourse.tile as tile
    from concourse import bacc

    def part_bcast(ap, parts):
        # replicate a [*free] AP across `parts` partitions (DMA replication)
        return bass.AP(tensor=ap.tensor, offset=ap.offset, ap=[[0, parts]] + list(ap.ap))

    F32 = mybir.dt.float32
    F32R = mybir.dt.float32r
    BF16 = mybir.dt.bfloat16
    EXP = mybir.ActivationFunctionType.Exp

    nc = bacc.Bacc("TRN2", target_bir_lowering=False, debug=False)

    x_d = nc.dram_tensor("x", [P, C], BF16, kind="ExternalInput")
    w_d = {
        nm: nc.dram_tensor(nm, [C, C], BF16, kind="ExternalInput")
        for nm in ("Wq", "Wk", "Wv", "Wo")
    }
    b_d = {
        nm: nc.dram_tensor(nm, [C], F32, kind="ExternalInput")
        for nm in ("bq", "bk", "bv", "bo")
    }
    out_d = nc.dram_tensor("out", [PQ, C], F32, kind="ExternalOutput")

    with tile.TileContext(nc) as tc, ExitStack() as ctx:
        const = ctx.enter_context(tc.tile_pool(name="const", bufs=1))
        big = ctx.enter_context(tc.tile_pool(name="big", bufs=1))
        ptiles = ctx.enter_context(tc.tile_pool(name="ptiles", bufs=2))
        stage = ctx.enter_context(tc.tile_pool(name="stage", bufs=3))
        small = ctx.enter_context(tc.tile_pool(name="small", bufs=4))

        ones_row = const.tile([1, 64], F32R, tag="ones_row")
        nc.gpsimd.memset(ones_row[:].bitcast(F32), 1.0)

        # touch exp early so the ACT table set loads during phase 1, not at
        # the first real softmax tile (table load is ~2.7us of ACT stall)
        warm = const.tile([1, 8], F32, tag="act_warm")
        nc.gpsimd.memset(warm, 0.0)
        nc.scalar.activation(out=warm, in_=warm, func=EXP)

        # weight/bias loads are deferred until after the first x-tile DMA is
        # enqueued, so the transpose pipeline starts as early as possible
        w_sb, bias_sb, bcast_sb = {}, {}, {}

        def load_weights():
            # on the scalar hwdge queue: the sync queue is in xbar-transpose
            # mode for the x loads, and mode transitions serialize a queue
            for nm in ("Wk", "Wq", "Wv", "Wo"):
                t = const.tile([128, 2, C], BF16, tag=f"w_{nm}", name=f"w_{nm}")
                for c2 in range(2):
                    nc.scalar.dma_start(
                        out=t[:, c2, :],
                        in_=w_d[nm][c2 * 128 : (c2 + 1) * 128, :],
                    )
                w_sb[nm] = t
            # per-partition bias layout for the channel-major Q^T/K^T tiles
            for nm in ("bq", "bk"):
                t = const.tile([128, 2], F32, tag=f"b_{nm}", name=f"b_{nm}")
                nc.scalar.dma_start(
                    out=t, in_=b_d[nm][:].rearrange("(c p) -> p c", p=128)
                )
                bias_sb[nm] = t
            # row-broadcast bias tiles for the row-major V / final projections
            for nm in ("bv", "bo"):
                t = const.tile([128, C], F32, tag=f"b_{nm}", name=f"b_{nm}")
                nc.gpsimd.dma_start(out=t, in_=part_bcast(b_d[nm][:], 128))
                bcast_sb[nm] = t

        xT = big.tile([128, 2, P], BF16, tag="xT")
        QT = big.tile([128, 2, PQ], BF16, tag="QT")
        KT = big.tile([128, 2, P], BF16, tag="KT")
        Vp = big.tile([128, NPT, H, D + 1], BF16, tag="Vp")
        OT = big.tile([128, 2, PQ], BF16, tag="OT")

        # ones column used by the PV matmul to accumulate softmax denominators
        nc.gpsimd.memset(Vp[:, :, :, D : D + 1], 1.0)

        # ---- phase 1: x^T, Q^T, K^T, V (interleaved per 512-row group) ----
        # x arrives bf16 and is transposed by the DMA xbar straight into xT
        # (no PE transposes, no DVE copies). After each 512-row group lands,
        # the K/Q projections for that group and the V projections for its
        # four 128-row tiles. Keeps the PE stream dense (HAM-warm) and gets
        # the first flash-loop inputs ready as early as possible.
        COPY = mybir.ActivationFunctionType.Identity
        with tc.tile_pool(name="ps_pj", bufs=2, space="PSUM") as ps_pj:
            def proj_kq(dst, w, bias, c2, mt):
                pp = ps_pj.tile([128, 512], F32, tag="proj", name="pp")
                for ci in range(2):
                    nc.tensor.matmul(
                        pp,
                        lhsT=w[:, ci, c2 * 128 : (c2 + 1) * 128],
                        rhs=xT[:, ci, mt * 512 : (mt + 1) * 512],
                        start=(ci == 0),
                        stop=(ci == 1),
                    )
                # evacuate on ACT (idle during phase 1; DVE is the phase-1
                # pacer): out = pp*1 + bias[c]
                nc.scalar.activation(
                    out=dst[:, c2, mt * 512 : (mt + 1) * 512],
                    in_=pp,
                    func=COPY,
                    bias=bias[:, c2 : c2 + 1],
                )

            for mt in range(P // 512):
                for c2 in range(2):
                    nc.sync.dma_start(
                        out=xT[:, c2, mt * 512 : (mt + 1) * 512],
                        in_=x_d[mt * 512 : (mt + 1) * 512, c2 * 128 : (c2 + 1) * 128],
                        transpose=True,
                    )
                if mt == 0:
                    load_weights()
                for c2 in range(2):
                    proj_kq(KT, w_sb["Wk"], bias_sb["bk"], c2, mt)
                    if mt < PQ // 512:
                        proj_kq(QT, w_sb["Wq"], bias_sb["bq"], c2, mt)
                for pt in range(4 * mt, 4 * mt + 4):
                    pv = ps_pj.tile([128, C], F32, tag="vproj", name="pv")
                    for ci in range(2):
                        nc.tensor.matmul(
                            pv,
                            lhsT=xT[:, ci, pt * 128 : (pt + 1) * 128],
                            rhs=w_sb["Wv"][:, ci, :],
                            start=(ci == 0),
                            stop=(ci == 1),
                        )
                    nc.vector.tensor_add(
                        out=Vp[:, pt, :, 0:D],
                        in0=pv.rearrange("p (h d) -> p h d", d=D),
                        in1=bcast_sb["bv"].rearrange("p (h d) -> p h d", d=D),
                    )
                if mt == 0:
                    # early HAM warm-up: dense back-to-back matmuls right after
                    # the first projection group unthrottle the PE for the rest
                    # of phase 1
                    with tc.tile_critical():
                        hamp = ps_pj.tile([128, 512], F32, tag="hamp", name="hamp")
                        for _ in range(10):
                            nc.tensor.matmul(
                                hamp,
                                lhsT=w_sb["Wk"][:, 0, 0:128],
                                rhs=xT[:, 0, 0:512],
                                start=True,
                                stop=True,
                            )

        # ---- phase 2: attention + output projection ----
        # The per-(m,pair) epilogue (reciprocal of the softmax denominator,
        # partition-broadcast, normalize, Wo projection, output DMA) has a
        # long serial latency chain (the [1,512] DVE reciprocal alone is
        # ~3.3us). If emitted in program order it stalls the PE at every
        # tile boundary, which re-throttles the HAM clock gate to 1.2 GHz
        # for the following stretch. Instead: evacuate the o PSUM banks
        # with a cheap DVE copy right away, then defer the rest of the
        # epilogue and emit it interleaved into the NEXT (m,pair)'s flash
        # loop so every engine always has ready work.
        with (
            tc.tile_pool(name="ps_s", bufs=1, space="PSUM") as ps_s,
            tc.tile_pool(name="ps_o", bufs=1, space="PSUM") as ps_o,
            tc.tile_pool(name="ps_w", bufs=1, space="PSUM") as ps_w,
            tc.tile_pool(name="obuf", bufs=2) as obuf,
        ):
            pending = []  # deferred epilogue thunks [(phase, fn), ...]

            def flush(phase):
                for ph, fn in [p for p in pending if p[0] == phase]:
                    fn()
                pending[:] = [p for p in pending if p[0] != phase]

            def epilogue_norm(m, ob_pair, heads, q0=0, q1=512):
                # reciprocal + broadcast + normalize into OT (DVE/PE work,
                # dependencies are several us old by the time this runs)
                qn = q1 - q0
                for j, h in enumerate(heads):
                    ob = ob_pair[j]
                    rc = small.tile([1, qn], F32R, tag="recip", name="rc")
                    with nc.allow_low_precision(reason="f32r recip ~1e-5"):
                        nc.vector.reciprocal(out=rc, in_=ob[D : D + 1, q0:q1])
                    bc = ps_w.tile([64, qn], F32, tag="rbc", name="bc")
                    nc.tensor.matmul(bc, lhsT=ones_row, rhs=rc, start=True, stop=True)
                    bcs = small.tile([64, qn], F32, tag="bcs", name="bcs")
                    nc.vector.tensor_copy(out=bcs, in_=bc)
                    bp, ch = 64 * (h % 2), h // 2
                    nc.vector.tensor_mul(
                        out=OT[bp : bp + 64, ch, m * 512 + q0 : m * 512 + q1],
                        in0=ob[0:D, q0:q1],
                        in1=bcs,
                    )

            def epilogue_wo(m, c0=0, c1=4):
                for pt4 in range(c0, c1):
                    pi = m * 4 + pt4
                    wp = ps_w.tile([128, C], F32, tag="wo", name="wp")
                    for ci in range(2):
                        nc.tensor.matmul(
                            wp,
                            lhsT=OT[:, ci, pi * 128 : (pi + 1) * 128],
                            rhs=w_sb["Wo"][:, ci, :],
                            start=(ci == 0),
                            stop=(ci == 1),
                        )
                    ot = stage.tile([128, C], F32, tag="outt", name="ot")
                    nc.vector.tensor_add(out=ot, in0=wp, in1=bcast_sb["bo"])
                    nc.sync.dma_start(out=out_d[pi * 128 : (pi + 1) * 128, :], in_=ot)

            NG = NPT // 2

            def emit_unit(m, heads, g):
                # one S+exp unit: 4 score matmuls (head pair row-grouped) for
                # key tiles 2g, 2g+1, plus the two exp activations
                s_ps = [
                    ps_s.tile([128, 2, 512], F32, tag=f"s{j}", name=f"s{j}")
                    for j in range(2)
                ]
                for j2 in range(2):
                    kt = 2 * g + j2
                    for j, h in enumerate(heads):
                        bp, ch = 64 * (h % 2), h // 2
                        nc.tensor.matmul(
                            s_ps[j][:, j2, :],
                            lhsT=KT[bp : bp + 64, ch, kt * 128 : (kt + 1) * 128],
                            rhs=QT[bp : bp + 64, ch, m * 512 : (m + 1) * 512],
                            start=True,
                            stop=True,
                        )
                p_sb = [
                    ptiles.tile([128, 2, 512], BF16, tag=f"p{j}", name=f"p{j}")
                    for j in range(2)
                ]
                for j in range(2):
                    nc.scalar.activation(out=p_sb[j], in_=s_ps[j], func=EXP, scale=SCALE)
                return p_sb

            # The S matmuls for unit i+2 are emitted BEFORE the PV matmuls of
            # unit i — across pair boundaries too — so in the PE's FIFO the
            # next scores are always queued ahead: the ACT engine never waits
            # for score tiles and runs gapless, which is the phase-2 critical
            # path.
            SEQ = [(m, pair) for m in range(PQ // 512) for pair in range(2)]
            TOT = len(SEQ) * NG

            def ctx_of(i):
                m, pair = SEQ[i // NG]
                return m, pair, i % NG

            def unit_of(i):
                m, pair, g = ctx_of(i)
                return emit_unit(m, (2 * pair, 2 * pair + 1), g)

            units = {0: unit_of(0), 1: unit_of(1)}
            o_ps = None
            for i in range(TOT):
                m, pair, g = ctx_of(i)
                heads = (2 * pair, 2 * pair + 1)
                p_sb = units.pop(i)
                if g == 0:
                    o_ps = [
                        ps_o.tile([D + 1, 512], F32, tag=f"o{j}", name=f"o{j}")
                        for j in range(2)
                    ]
                if i == 3:
                    # ~5us of dense back-to-back matmuls: guarantees a
                    # fully-busy HAM SHORT window so the PE un-throttles to
                    # 2.4 GHz early. lhsT depends on this unit's exp output,
                    # which pins the burst into the loop's timeline; the
                    # critical section keeps it contiguous (the scheduler
                    # would otherwise sink this no-consumer work to the end).
                    with tc.tile_critical():
                        hamw = ps_w.tile([128, 512], F32, tag="rbc", name="hamw")
                        for _ in range(12):
                            nc.tensor.matmul(
                                hamw,
                                lhsT=p_sb[0][:, 0, 0:128],
                                rhs=xT[:, 0, 0:512],
                                start=True,
                                stop=True,
                            )
                if g == 4:
                    flush("norm")
                if g == 8:
                    flush("wo")
                if g in (0, 1, 14, 15) and i > 3:
                    # pair-boundary HAM pad: keeps the PE idle window at the
                    # epilogue handoff below the re-throttle threshold.
                    padw = ps_w.tile([128, 512], F32, tag="rbc", name=f"padw{g}")
                    for _ in range(3):
                        nc.tensor.matmul(
                            padw,
                            lhsT=p_sb[0][:, 0, 0:128],
                            rhs=xT[:, 0, 0:512],
                            start=True,
                            stop=True,
                        )
                for j2 in range(2):
                    kt = 2 * g + j2
                    for j, h in enumerate(heads):
                        nc.tensor.matmul(
                            o_ps[j],
                            lhsT=Vp[:, kt, h, :],
                            rhs=p_sb[j][:, j2, :],
                            start=(kt == 0),
                            stop=(kt == NPT - 1),
                            skip_group_check=True,
                        )
                if i + 2 < TOT:
                    units[i + 2] = unit_of(i + 2)
                if g == NG - 1:
                    # evacuate o PSUM banks immediately (cheap copies) so the
                    # next pair's PV matmuls are never blocked on the slow
                    # normalization chain
                    ob_pair = [
                        obuf.tile([D + 1, 512], F32, tag=f"ob{j}", name=f"ob{j}")
                        for j in range(2)
                    ]
                    for j in range(2):
                        nc.vector.tensor_copy(out=ob_pair[j], in_=o_ps[j])
                    if i == TOT - 1:
                        # final pair: emit the epilogue chunked by query-half
                        # so the reciprocal / broadcast / Wo / DMA chains of
                        # the two halves pipeline, shortening the kernel tail
                        for half in range(2):
                            epilogue_norm(m, ob_pair, heads, half * 256, (half + 1) * 256)
                            epilogue_wo(m, half * 2, (half + 1) * 2)
                    else:
                        pending.append(
                            ("norm", (lambda mm, op, hh: lambda: epilogue_norm(mm, op, hh))(m, ob_pair, heads))
                        )
                        if pair == 1:
                            pending.append(("wo", (lambda mm: lambda: epilogue_wo(mm))(m)))
            flush("norm")
            flush("wo")

    nc.compile()
    return nc


def _get_nc():
    if "nc" not in _CACHE:
        _CACHE["nc"] = _build()
    return _CACHE["nc"]


def _in_maps(inputs):
    import ml_dtypes

    x = np.ascontiguousarray(
        np.asarray(inputs["x"], dtype=np.float32).astype(ml_dtypes.bfloat16)
    )
    assert x.shape == (B, P, C), x.shape
    shared = {}
    for nm in ("bq", "bk", "bv", "bo"):
        shared[nm] = np.ascontiguousarray(np.asarray(inputs[nm], dtype=np.float32))
    for nm in ("Wq", "Wk", "Wv", "Wo"):
        shared[nm] = np.ascontiguousarray(
            np.asarray(inputs[nm], dtype=np.float32).astype(ml_dtypes.bfloat16)
        )
    maps = []
    for core in range(N_CORES):
        b, half = core // 2, core % 2
        if half == 0:
            xl = np.ascontiguousarray(x[b])
        else:
            xl = np.ascontiguousarray(np.roll(x[b], -PQ, axis=0))
        maps.append({"x": xl, **shared})
    return maps


def run(inputs, trace=False):
    from concourse import bass_utils

    nc = _get_nc()
    res = bass_utils.run_bass_kernel_spmd(
        nc, _in_maps(inputs), core_ids=list(range(N_CORES)), trace=trace
    )
    out = np.empty((B, P, C), np.float32)
    for core in range(N_CORES):
        b, half = core // 2, core % 2
        out[b, half * PQ : (half + 1) * PQ] = res.results[core]["out"]
    return out, res


def kernel(**inputs):
    out, _ = run(inputs, trace=False)
    return out

